# revision 14
# baseline (speedup 1.0000x reference)
"""Causal self-attention (B=4, T=2048, C=1024, H=16, D=64) on 8 TRN2 NeuronCores.

Sharding: core c handles batch b = c//2 and head-group hg = c%2 (8 of 16 heads).
Per core: column-sharded QKV projection (only its heads' q/k/v columns, only its
batch's rows), full causal attention for its 8 heads, row-sharded output
projection producing a partial [T, C] result. Host sums the two head-group
partials per batch (the "all-reduce") and adds the bias correction term.

Math notes:
 - k-bias is dropped: softmax((q+bq)@(k+bk)^T) == softmax((q+bq)@k^T) because
   the (q+bq)@bk term is constant along the key axis.
 - v-bias and proj-bias are folded into a host-side correction: since softmax
   rows sum to 1, y = P@(V + 1 bv^T) = P@V + 1 bv^T, so the output correction
   is bv @ w_proj + b_proj added to every row.
 - Attention works in S^T layout ([keys, q]): softmax denominators come from a
   ones-column appended to V (row 64 of the wide PV accumulation
   y^T[65,512] += V'.T @ expS), then y^T is transposed back 128 columns at a
   time on the PE so the normalization is a per-partition scalar multiply.

Schedule: the three stages (QKV projection, attention, output projection) are
interleaved per 512-row slab r5 — attention for query block q5=r5 only needs
rows <= (r5+1)*512 of q/k/v, and the output projection for those rows only
needs attention outputs for them. This keeps the ACT engine's exp stream (the
attention bottleneck) overlapped with the PE-heavy projection matmuls. All
PSUM users except the PV accumulators share one 3-deep ring of 2-bank tiles
(tag "big"); matmul accumulation groups are bank-granular (start clears the
whole 2KB zero-region), so multi-group banks start only on their first write.
"""

import numpy as np
import ml_dtypes

B, T, C, H, D = 4, 2048, 1024, 16, 64
HC = 8            # heads per core
KCH = C // 128    # 8 contraction chunks
RC = T // 128     # 16 row chunks
QQ = T // 512     # 4 query super-blocks
BF16 = ml_dtypes.bfloat16

_COMPILED = {}


def _build_nc():
    from concourse import bacc
    import concourse.tile as tile
    from concourse import mybir
    from concourse.masks import make_identity

    bf16 = mybir.dt.bfloat16
    f32 = mybir.dt.float32
    EXP = mybir.ActivationFunctionType.Exp
    CPY = mybir.ActivationFunctionType.Copy
    IDN = mybir.ActivationFunctionType.Identity

    nc = bacc.Bacc(None, target_bir_lowering=False)

    xT = nc.dram_tensor("xT", [128, KCH, T], bf16, kind="ExternalInput")
    wqk = nc.dram_tensor("wqk", [128, KCH, 8, 128], bf16, kind="ExternalInput")
    wv = nc.dram_tensor("wv", [128, KCH, 512], bf16, kind="ExternalInput")
    bq = nc.dram_tensor("bq", [128, 4], f32, kind="ExternalInput")  # pre-scaled
    wp = nc.dram_tensor("wp", [128, 4, 1024], bf16, kind="ExternalInput")
    out = nc.dram_tensor("out", [T, C], f32, kind="ExternalOutput")

    # Causal mask for the diagonal 128-key x 512-q blocks, variant r = kc % 4:
    # valid iff r*128 + k <= q. Multiplied into exp(S) in bf16 on GpSimd.
    kk = np.arange(128)[:, None, None]
    rr = np.arange(4)[None, :, None]
    qq = np.arange(512)[None, None, :]
    mask_np = (rr * 128 + kk <= qq).astype(BF16)
    msk = nc.inline_tensor(mask_np, name="msk")

    with tile.TileContext(nc) as tc:
        with tc.tile_pool(name="singles", bufs=1) as singles:
            wqk_sb = singles.tile([128, KCH, 8, 128], bf16)
            wv_sb = singles.tile([128, KCH, 512], bf16)
            bq_sb = singles.tile([128, 4], f32)
            wp_sb = singles.tile([128, 4, 1024], bf16)
            msk_sb = singles.tile([128, 4, 512], bf16)
            ident = singles.tile([128, 128], bf16)
            identf = singles.tile([128, 128], f32)

            # persistent activations
            qT_sb = singles.tile([128, 4, T], bf16)   # q^T, heads 2c,2c+1 in chunk c
            kT_sb = singles.tile([128, 4, T], bf16)
            v_sb = singles.tile([128, RC, HC, 65], bf16)  # V natural + ones col
            y_sb = singles.tile([128, RC, 512], bf16)     # attention out, natural

            # input DMAs: weights needed first on the SP ring; the rest on the
            # ACT ring so they don't delay the first matmuls
            nc.sync.dma_start(wqk_sb[:], wqk[:])
            nc.sync.dma_start(wv_sb[:], wv[:])
            nc.scalar.dma_start(bq_sb[:], bq[:])
            nc.scalar.dma_start(msk_sb[:], msk[:])
            nc.scalar.dma_start(wp_sb[:], wp[:])
            make_identity(nc, ident[:])
            make_identity(nc, identf[:])
            nc.gpsimd.memset(v_sb[:, :, :, 64], 1.0)

            with tc.tile_pool(name="xt", bufs=2) as xp, \
                 tc.tile_pool(name="att", bufs=3) as ap_, \
                 tc.tile_pool(name="attn", bufs=8) as anp, \
                 tc.tile_pool(name="proj", bufs=2) as pp, \
                 tc.tile_pool(name="outp", bufs=2) as op_, \
                 tc.tile_pool(name="big", bufs=3, space="PSUM") as big, \
                 tc.tile_pool(name="psYT", bufs=1, space="PSUM") as psYT:

                for r5 in range(4):
                    sl = slice(r5 * 512, (r5 + 1) * 512)

                    # ---- QKV projection for row slab r5 ----
                    xt = xp.tile([128, KCH, 512], bf16)
                    nc.sync.dma_start(xt[:], xT[:, :, sl])
                    for ccp in range(4):          # qk col-chunk pairs
                        ps = big.tile([128, 2, 512], f32, tag="big", name="psqk")
                        for half in range(2):
                            cc = ccp * 2 + half
                            for kc in range(KCH):
                                nc.tensor.matmul(
                                    ps[:, half, :], wqk_sb[:, kc, cc, :],
                                    xt[:, kc, :],
                                    start=(kc == 0), stop=(kc == KCH - 1))
                        for half in range(2):
                            cc = ccp * 2 + half
                            if cc < 4:   # q columns: scale+bias, on ACT
                                nc.scalar.activation(
                                    out=qT_sb[:, cc, sl], in_=ps[:, half, :],
                                    func=IDN, scale=0.125,
                                    bias=bq_sb[:, cc:cc + 1])
                            else:        # k columns: plain copy, on ACT
                                nc.scalar.activation(
                                    out=kT_sb[:, cc - 4, sl], in_=ps[:, half, :],
                                    func=CPY)
                    for rp in range(2):           # v row pairs (128 rows each)
                        psv = big.tile([128, 2, 512], f32, tag="big", name="psv")
                        for half in range(2):
                            rs = rp * 2 + half
                            for kc in range(KCH):
                                nc.tensor.matmul(
                                    psv[:, half, :],
                                    xt[:, kc, rs * 128:(rs + 1) * 128],
                                    wv_sb[:, kc, :],
                                    start=(kc == 0), stop=(kc == KCH - 1))
                        for half in range(2):
                            rc = r5 * 4 + rp * 2 + half
                            nc.vector.tensor_copy(
                                out=v_sb[:, rc, :, 0:64],
                                in_=psv[:, half, :].rearrange(
                                    "p (h d) -> p h d", h=HC))

                    # ---- attention for query block q5 = r5 ----
                    q5 = r5
                    nkc = 4 * (q5 + 1)
                    for pr in range(4):           # head pair: 2pr, 2pr+1
                        psyt = [psYT.tile([65, 512], f32, name=f"psyt{i}")
                                for i in range(2)]
                        for kc in range(nkc):
                            diag = (kc // 4 == q5)
                            r = kc % 4
                            qof = r * 128 if diag else 0   # causal column trim
                            pss = big.tile([128, 2, 512], f32, tag="big",
                                           name="pss")
                            for i in range(2):    # heads packed in the PE
                                po = i * 64
                                nc.tensor.matmul(
                                    pss[:, i, qof:],
                                    kT_sb[po:po + 64, pr, kc * 128:(kc + 1) * 128],
                                    qT_sb[po:po + 64, pr,
                                          q5 * 512 + qof:(q5 + 1) * 512],
                                    start=True, stop=True)
                            exps = ap_.tile([128, 2, 512], bf16)
                            nc.scalar.activation(exps[:, :, qof:],
                                                 pss[:, :, qof:], EXP)
                            for i in range(2):
                                h = 2 * pr + i
                                if diag:
                                    # only the 128-col triangle block needs
                                    # masking; columns past it are fully valid
                                    nc.vector.tensor_mul(
                                        out=exps[:, i, qof:qof + 128],
                                        in0=exps[:, i, qof:qof + 128],
                                        in1=msk_sb[:, r, qof:qof + 128])
                                nc.tensor.matmul(
                                    psyt[i][:, qof:], v_sb[:, kc, h, :],
                                    exps[:, i, qof:],
                                    start=(kc == 0), stop=(kc == nkc - 1))
                        for i in range(2):
                            h = 2 * pr + i
                            ytf = ap_.tile([65, 512], f32, name="ytf")
                            nc.vector.tensor_copy(out=ytf[:], in_=psyt[i][:])
                            ytt = big.tile([128, 2, 512], f32, tag="big",
                                           name="ytt")
                            for j in range(4):
                                # transposes share one bank: start on first only
                                nc.tensor.matmul(
                                    ytt[:, 0, j * 65:(j + 1) * 65],
                                    ytf[:, j * 128:(j + 1) * 128],
                                    identf[0:65, 0:65], is_transpose=True,
                                    start=(j == 0), stop=(j == 3),
                                    skip_group_check=True)
                            for j in range(4):
                                rc = q5 * 4 + j
                                recip = anp.tile([128, 1], f32)
                                nc.vector.reciprocal(
                                    recip[:], ytt[:, 0, j * 65 + 64:j * 65 + 65])
                                nc.vector.tensor_scalar_mul(
                                    out=y_sb[:, rc, h * 64:(h + 1) * 64],
                                    in0=ytt[:, 0, j * 65:j * 65 + 64],
                                    scalar1=recip[:])

                    # ---- output projection for row slab r5 ----
                    for rs in range(4):
                        rc = r5 * 4 + rs
                        ptt = big.tile([128, 2, 512], bf16, tag="big",
                                       name="ptt")
                        for t in range(4):
                            nc.tensor.matmul(
                                ptt[:, 0, t * 128:(t + 1) * 128],
                                y_sb[:, rc, t * 128:(t + 1) * 128],
                                ident[:], is_transpose=True,
                                start=(t == 0), stop=(t == 3),
                                skip_group_check=True)
                        yT = pp.tile([128, 4, 128], bf16)
                        nc.vector.tensor_copy(
                            out=yT[:], in_=ptt[:, 0, :].rearrange(
                                "p (t w) -> p t w", t=4))
                        pso = big.tile([128, 2, 512], f32, tag="big",
                                       name="pso")
                        for t in range(4):
                            for oh in range(2):
                                nc.tensor.matmul(
                                    pso[:, oh, :], yT[:, t, :],
                                    wp_sb[:, t, oh * 512:(oh + 1) * 512],
                                    start=(t == 0), stop=(t == 3),
                                    skip_group_check=True)
                        osb = op_.tile([128, 1024], f32)
                        nc.vector.tensor_copy(
                            out=osb[:].rearrange("p (a b) -> p a b", a=2),
                            in_=pso[:])
                        nc.scalar.dma_start(out[rc * 128:(rc + 1) * 128, :],
                                            osb[:])

    nc.compile()
    return nc


def _prep_core_inputs(x, w_attn, b_attn, w_proj, c):
    b, hg = c // 2, c % 2
    xb = np.ascontiguousarray(x[b])                       # [T, C]
    xT = np.ascontiguousarray(
        xb.T.reshape(KCH, 128, T).transpose(1, 0, 2)).astype(BF16)
    wq = w_attn[:, hg * 512:(hg + 1) * 512]
    wk = w_attn[:, C + hg * 512:C + (hg + 1) * 512]
    wqk = np.concatenate([wq, wk], axis=1)                # [C, 1024]
    wqk = np.ascontiguousarray(
        wqk.reshape(KCH, 128, 8, 128).transpose(1, 0, 2, 3)).astype(BF16)
    wv = w_attn[:, 2 * C + hg * 512:2 * C + (hg + 1) * 512]
    wv = np.ascontiguousarray(
        wv.reshape(KCH, 128, 512).transpose(1, 0, 2)).astype(BF16)
    bqv = np.ascontiguousarray(
        (0.125 * b_attn[hg * 512:(hg + 1) * 512]).reshape(4, 128).T
    ).astype(np.float32)
    wpc = w_proj[hg * 512:(hg + 1) * 512, :]
    wpc = np.ascontiguousarray(
        wpc.reshape(4, 128, 1024).transpose(1, 0, 2)).astype(BF16)
    return {"xT": xT, "wqk": wqk, "wv": wv, "bq": bqv, "wp": wpc}


def _run(nc, in_maps, **kwargs):
    from concourse.bass_utils import run_bass_kernel_spmd
    return run_bass_kernel_spmd(nc, in_maps, core_ids=list(range(8)), **kwargs)


def kernel(x, w_attn, b_attn, w_proj, b_proj, _trace=False):
    x = np.asarray(x, dtype=np.float32)
    w_attn = np.asarray(w_attn, dtype=np.float32)
    b_attn = np.asarray(b_attn, dtype=np.float32)
    w_proj = np.asarray(w_proj, dtype=np.float32)
    b_proj = np.asarray(b_proj, dtype=np.float32)

    if "nc" not in _COMPILED:
        _COMPILED["nc"] = _build_nc()
    nc = _COMPILED["nc"]

    in_maps = [_prep_core_inputs(x, w_attn, b_attn, w_proj, c) for c in range(8)]
    kwargs = {"trace": True} if _trace else {}
    res = _run(nc, in_maps, **kwargs)
    _COMPILED["last_result"] = res

    corr = b_attn[2 * C:].astype(np.float32) @ w_proj + b_proj
    out = np.empty((B, T, C), np.float32)
    for b in range(B):
        out[b] = res.results[2 * b]["out"] + res.results[2 * b + 1]["out"]
        out[b] += corr[None, :]
    return out


# revision 18
# speedup vs baseline: 1.3275x; 1.3275x over previous
"""Causal self-attention (B=4, T=2048, C=1024, H=16, D=64) on 8 TRN2 NeuronCores.

Sharding: core c handles batch b = c//2 and head-group hg = c%2 (8 of 16 heads).
Per core: column-sharded QKV projection (only its heads' q/k/v columns, only its
batch's rows), full causal attention for its 8 heads, row-sharded output
projection producing a partial [T, C] result. Host sums the two head-group
partials per batch (the "all-reduce") and adds the bias correction term.

Math notes:
 - k-bias is dropped: softmax((q+bq)@(k+bk)^T) == softmax((q+bq)@k^T) because
   the (q+bq)@bk term is constant along the key axis.
 - v-bias and proj-bias are folded into a host-side correction: since softmax
   rows sum to 1, y = P@(V + 1 bv^T) = P@V + 1 bv^T, so the output correction
   is bv @ w_proj + b_proj added to every row.
 - Attention works in S^T layout ([keys, q]): softmax denominators come from a
   ones-column appended to V (row 64 of the PV accumulation), and the PV
   matmul P^T.T @ V' = P @ V' lands y in natural [q, d] layout so the
   normalization is a per-partition scalar multiply.
"""

import numpy as np
import ml_dtypes

B, T, C, H, D = 4, 2048, 1024, 16, 64
HC = 8            # heads per core
KCH = C // 128    # 8 contraction chunks
RC = T // 128     # 16 row chunks
QQ = T // 512     # 4 query super-blocks
BF16 = ml_dtypes.bfloat16

_COMPILED = {}


def _build_nc():
    from concourse import bacc
    import concourse.tile as tile
    from concourse import mybir
    from concourse.masks import make_identity

    bf16 = mybir.dt.bfloat16
    f32 = mybir.dt.float32
    EXP = mybir.ActivationFunctionType.Exp
    ADD = mybir.AluOpType.add
    MULT = mybir.AluOpType.mult

    nc = bacc.Bacc(None, target_bir_lowering=False)

    xT = nc.dram_tensor("xT", [128, KCH, T], bf16, kind="ExternalInput")
    wqk = nc.dram_tensor("wqk", [128, KCH, 8, 128], bf16, kind="ExternalInput")
    wv = nc.dram_tensor("wv", [128, KCH, 512], bf16, kind="ExternalInput")
    bq = nc.dram_tensor("bq", [128, 4], f32, kind="ExternalInput")
    wp = nc.dram_tensor("wp", [128, 4, 1024], bf16, kind="ExternalInput")
    out = nc.dram_tensor("out", [T, C], f32, kind="ExternalOutput")

    # Causal mask for the diagonal 128-key x 512-q blocks, variant r = kc % 4:
    # valid iff r*128 + k <= q. Applied multiplicatively to exp(S) in bf16.
    kk = np.arange(128)[:, None, None]
    rr = np.arange(4)[None, :, None]
    qq = np.arange(512)[None, None, :]
    mask_np = (rr * 128 + kk <= qq).astype(BF16)
    msk = nc.inline_tensor(mask_np, name="msk")

    with tile.TileContext(nc) as tc:
        with tc.tile_pool(name="singles", bufs=1) as singles:
            wqk_sb = singles.tile([128, KCH, 8, 128], bf16)
            wv_sb = singles.tile([128, KCH, 512], bf16)
            bq_sb = singles.tile([128, 4], f32)
            wp_sb = singles.tile([128, 4, 1024], bf16)
            msk_sb = singles.tile([128, 4, 512], bf16)
            ident = singles.tile([128, 128], bf16)
            identf = singles.tile([128, 128], f32)
            # weights needed first go on the SP ring; the rest on the ACT ring
            # so they don't delay the first projection matmuls
            nc.sync.dma_start(wqk_sb[:], wqk[:])
            nc.sync.dma_start(wv_sb[:], wv[:])
            nc.scalar.dma_start(bq_sb[:], bq[:])
            nc.scalar.dma_start(msk_sb[:], msk[:])
            nc.scalar.dma_start(wp_sb[:], wp[:])
            make_identity(nc, ident[:])
            make_identity(nc, identf[:])

            # persistent activations
            qT_sb = singles.tile([128, 4, T], bf16)   # q^T, heads 2c,2c+1 in chunk c
            kT_sb = singles.tile([128, 4, T], bf16)
            v_sb = singles.tile([128, RC, HC, 65], bf16)  # V natural + ones col
            y_sb = singles.tile([128, RC, 512], bf16)     # attention output, natural

            nc.vector.memset(v_sb[:, :, :, 64], 1.0)

            # ---- Phase 1: QKV projection ----
            with tc.tile_pool(name="xt", bufs=3) as xp, \
                 tc.tile_pool(name="psA", bufs=4, space="PSUM") as psA:
                for r5 in range(4):           # 512-row chunks
                    sl = slice(r5 * 512, (r5 + 1) * 512)
                    xt = xp.tile([128, KCH, 512], bf16)
                    nc.sync.dma_start(xt[:], xT[:, :, sl])
                    for cc in range(8):       # qk column chunks (0-3 q, 4-7 k)
                        ps = psA.tile([128, 512], f32)
                        for kc in range(KCH):
                            nc.tensor.matmul(ps[:], wqk_sb[:, kc, cc, :],
                                             xt[:, kc, :],
                                             start=(kc == 0), stop=(kc == KCH - 1))
                        if cc < 4:
                            nc.vector.tensor_scalar(
                                out=qT_sb[:, cc, sl], in0=ps[:],
                                scalar1=bq_sb[:, cc:cc + 1], scalar2=0.125,
                                op0=ADD, op1=MULT)
                        else:
                            nc.vector.tensor_copy(out=kT_sb[:, cc - 4, sl], in_=ps[:])
                    for rs in range(4):       # v rows, 128 at a time
                        rc = r5 * 4 + rs
                        psv = psA.tile([128, 512], f32)
                        for kc in range(KCH):
                            nc.tensor.matmul(psv[:], xt[:, kc, rs * 128:(rs + 1) * 128],
                                             wv_sb[:, kc, :],
                                             start=(kc == 0), stop=(kc == KCH - 1))
                        nc.vector.tensor_copy(
                            out=v_sb[:, rc, :, 0:64],
                            in_=psv[:].rearrange("p (h d) -> p h d", h=HC))

            # ---- Phase 2: attention (head pairs packed via tile_position) ----
            # S^T = K^T.T @ Q^T per 128-key chunk (two heads packed in the PE
            # array); exp on ACT; causal mask multiplied into exp(S) in bf16;
            # wide PV: y^T[65,512] += V'.T @ expS with V' stationary; then
            # transpose y^T back to natural layout and normalize by the
            # ones-column row sums.
            with tc.tile_pool(name="att", bufs=3) as ap, \
                 tc.tile_pool(name="attn", bufs=8) as anp, \
                 tc.tile_pool(name="psS", bufs=2, space="PSUM") as psS, \
                 tc.tile_pool(name="psYT", bufs=1, space="PSUM") as psYT, \
                 tc.tile_pool(name="psTT", bufs=2, space="PSUM") as psTT:
                for pr in range(4):           # head pair: local heads 2pr, 2pr+1
                    for q5 in range(QQ):
                        qsl = slice(q5 * 512, (q5 + 1) * 512)
                        psyt = [psYT.tile([65, 512], f32, name=f"psyt{i}")
                                for i in range(2)]
                        nkc = 4 * (q5 + 1)
                        for kc in range(nkc):
                            diag = (kc // 4 == q5)
                            r = kc % 4
                            qof = r * 128 if diag else 0  # causal column trim
                            pss = psS.tile([128, 2, 512], f32)
                            for i in range(2):   # head in pair, packed in PE
                                po = i * 64
                                nc.tensor.matmul(
                                    pss[:, i, qof:],
                                    kT_sb[po:po + 64, pr, kc * 128:(kc + 1) * 128],
                                    qT_sb[po:po + 64, pr,
                                          q5 * 512 + qof:(q5 + 1) * 512],
                                    start=True, stop=True)
                            exps = ap.tile([128, 2, 512], bf16)
                            nc.scalar.activation(exps[:, :, qof:],
                                                 pss[:, :, qof:], EXP)
                            for i in range(2):
                                h = 2 * pr + i
                                if diag:
                                    # only the 128-col triangle needs masking;
                                    # columns past it are fully valid
                                    nc.vector.tensor_mul(
                                        out=exps[:, i, qof:qof + 128],
                                        in0=exps[:, i, qof:qof + 128],
                                        in1=msk_sb[:, r, qof:qof + 128])
                                nc.tensor.matmul(
                                    psyt[i][:, qof:], v_sb[:, kc, h, :],
                                    exps[:, i, qof:],
                                    start=(kc == 0), stop=(kc == nkc - 1))
                        for i in range(2):
                            h = 2 * pr + i
                            ytf = ap.tile([65, 512], f32, name="ytf")
                            nc.vector.tensor_copy(out=ytf[:], in_=psyt[i][:])
                            for j in range(4):
                                rc = q5 * 4 + j
                                pst = psTT.tile([128, 65], f32)
                                nc.tensor.transpose(
                                    pst[:], ytf[:, j * 128:(j + 1) * 128],
                                    identf[0:65, 0:65])
                                recip = anp.tile([128, 1], f32)
                                nc.vector.reciprocal(recip[:], pst[:, 64:65])
                                nc.vector.tensor_scalar_mul(
                                    out=y_sb[:, rc, h * 64:(h + 1) * 64],
                                    in0=pst[:, 0:64], scalar1=recip[:])

            # ---- Phase 3: output projection ----
            with tc.tile_pool(name="proj", bufs=3) as pp, \
                 tc.tile_pool(name="outp", bufs=3) as op_, \
                 tc.tile_pool(name="psT", bufs=3, space="PSUM") as psT, \
                 tc.tile_pool(name="psO", bufs=2, space="PSUM") as psO:
                for rc in range(RC):
                    yT = pp.tile([128, 4, 128], bf16)
                    for t in range(4):
                        pst = psT.tile([128, 128], bf16)
                        nc.tensor.transpose(pst[:], y_sb[:, rc, t * 128:(t + 1) * 128],
                                            ident[:])
                        nc.vector.tensor_copy(out=yT[:, t, :], in_=pst[:])
                    pso = psO.tile([128, 2, 512], f32)
                    for t in range(4):
                        for oh in range(2):
                            nc.tensor.matmul(pso[:, oh, :], yT[:, t, :],
                                             wp_sb[:, t, oh * 512:(oh + 1) * 512],
                                             start=(t == 0), stop=(t == 3),
                                             skip_group_check=True)
                    osb = op_.tile([128, 1024], f32)
                    nc.vector.tensor_copy(
                        out=osb[:].rearrange("p (a b) -> p a b", a=2), in_=pso[:])
                    nc.scalar.dma_start(out[rc * 128:(rc + 1) * 128, :], osb[:])

    nc.compile()
    return nc


def _prep_core_inputs(x, w_attn, b_attn, w_proj, c):
    b, hg = c // 2, c % 2
    xb = np.ascontiguousarray(x[b])                       # [T, C]
    xT = np.ascontiguousarray(
        xb.T.reshape(KCH, 128, T).transpose(1, 0, 2)).astype(BF16)
    wq = w_attn[:, hg * 512:(hg + 1) * 512]
    wk = w_attn[:, C + hg * 512:C + (hg + 1) * 512]
    wqk = np.concatenate([wq, wk], axis=1)                # [C, 1024]
    wqk = np.ascontiguousarray(
        wqk.reshape(KCH, 128, 8, 128).transpose(1, 0, 2, 3)).astype(BF16)
    wv = w_attn[:, 2 * C + hg * 512:2 * C + (hg + 1) * 512]
    wv = np.ascontiguousarray(
        wv.reshape(KCH, 128, 512).transpose(1, 0, 2)).astype(BF16)
    bqv = np.ascontiguousarray(
        b_attn[hg * 512:(hg + 1) * 512].reshape(4, 128).T).astype(np.float32)
    wpc = w_proj[hg * 512:(hg + 1) * 512, :]
    wpc = np.ascontiguousarray(
        wpc.reshape(4, 128, 1024).transpose(1, 0, 2)).astype(BF16)
    return {"xT": xT, "wqk": wqk, "wv": wv, "bq": bqv, "wp": wpc}


def _run(nc, in_maps, **kwargs):
    from concourse.bass_utils import run_bass_kernel_spmd
    return run_bass_kernel_spmd(nc, in_maps, core_ids=list(range(8)), **kwargs)


def kernel(x, w_attn, b_attn, w_proj, b_proj, _trace=False):
    x = np.asarray(x, dtype=np.float32)
    w_attn = np.asarray(w_attn, dtype=np.float32)
    b_attn = np.asarray(b_attn, dtype=np.float32)
    w_proj = np.asarray(w_proj, dtype=np.float32)
    b_proj = np.asarray(b_proj, dtype=np.float32)

    if "nc" not in _COMPILED:
        _COMPILED["nc"] = _build_nc()
    nc = _COMPILED["nc"]

    in_maps = [_prep_core_inputs(x, w_attn, b_attn, w_proj, c) for c in range(8)]
    kwargs = {"trace": True} if _trace else {}
    res = _run(nc, in_maps, **kwargs)
    _COMPILED["last_result"] = res

    corr = b_attn[2 * C:].astype(np.float32) @ w_proj + b_proj
    out = np.empty((B, T, C), np.float32)
    for b in range(B):
        out[b] = res.results[2 * b]["out"] + res.results[2 * b + 1]["out"]
        out[b] += corr[None, :]
    return out
